# revision 12
# baseline (speedup 1.0000x reference)
"""Differentiable Voronoi propagation on 8 Trainium2 NeuronCores.

Strategy (C-sharded): 196 clusters padded to 200, 25 per core. Each core
propagates its distance maps D = ALPHA*dist for 20 iterations fully
SBUF-resident, then the softmax over all 200 clusters is computed with two
small AllReduces (per-pixel min, per-pixel sum).

Per-core layout: image row r = 2p + b  (p in [0,112) partitions, b in {0,1});
SBUF free dim = (cluster c, parity b, stored col ws) with ws = [wrapL, w0..w223,
wrapR] so E/W rolls are free-dim offsets. N/S rolls decompose into a free-dim
parity swap plus a circular partition shift done as an exact fp32 permutation
matmul on the TensorEngine (PSUM), everything else on VectorE/GpSimd/ScalarE:
cand_i = roll_i(D)+10*cost_i, m = chained min, e_i = Exp(-(cand_i-m)) in fp16,
D = m - Ln(sum e_i).
"""
import numpy as np

import concourse.bass as bass
import concourse.mybir as mybir
import concourse.tile as tile
from concourse.bass_utils import run_bass_kernel_spmd

B, C, H, W = 1, 196, 224, 224
NUM_ITERS = 20
ALPHA = 10.0
BIG = 1.0e4
DEAD = 3.0e5
CPAD, NCORE = 200, 8
CPC = CPAD // NCORE      # 25 clusters per core
P = 112                  # partitions (= H/2)
WS = W + 2               # stored row width incl wrap cols
FREE_D = CPC * 2 * WS    # 11300
FREE_O = CPC * 2 * W     # 11200
CCH = 2                  # clusters per chunk
DIRS = ((-1, 0), (1, 0), (0, -1), (0, 1))

AF = mybir.ActivationFunctionType
ALU = mybir.AluOpType
F32 = mybir.dt.float32
F16 = mybir.dt.float16

_PROG = None


# ----------------------------------------------------------------- host prep
def _host_costs(x, edge_weight):
    x = np.asarray(x, np.float32)
    ew = np.asarray(edge_weight, np.float32)
    gray = (0.2989 * x[:, 0] + 0.587 * x[:, 1] + 0.114 * x[:, 2])[0]
    gp = np.pad(gray, 1)
    edges = np.zeros((2, H, W), np.float32)
    for o in range(2):
        k = ew[o, 0]
        acc = np.zeros((H, W), np.float32)
        for i in range(3):
            for j in range(3):
                acc += k[i, j] * gp[i:i + H, j:j + W]
        edges[o] = acc
    grad = np.sqrt(edges[0] ** 2 + edges[1] ** 2)
    wgrad = grad ** 4 * np.float32(10.0)
    costs = np.zeros((4, H, W), np.float32)
    for i, (dy, dx) in enumerate(DIRS):
        rolled = np.roll(x[0], (dy, dx), axis=(1, 2))
        costs[i] = wgrad + np.float32(10.0) * np.abs(x[0] - rolled).sum(0)
    return (np.float32(ALPHA) * costs).astype(np.float32)


def _to_layout(plane):
    """(..., H, W) -> (..., P, 2, WS) with wrap cols; row r = 2p + b."""
    sh = plane.shape[:-2]
    out = np.zeros(sh + (P, 2, WS), plane.dtype)
    r = plane.reshape(sh + (P, 2, W))
    out[..., 1:W + 1] = r
    out[..., 0] = r[..., W - 1]
    out[..., W + 1] = r[..., 0]
    return out


def _host_inputs(x, edge_weight, int_cy, int_cx):
    costs10 = _host_costs(x, edge_weight)
    costs_l = _to_layout(costs10)                      # (4,P,2,WS)
    # replicate over a 2-cluster axis: layout [P, (cr=2, d=4, b=2, ws)]
    rep = np.broadcast_to(costs_l.transpose(1, 0, 2, 3)[:, None],
                          (P, 2, 4, 2, WS))
    costs_in = np.ascontiguousarray(rep.reshape(P, 2 * 4 * 2 * WS))
    cy = np.asarray(int_cy); cx = np.asarray(int_cx)
    d0 = np.full((CPAD, H, W), np.float32(ALPHA * BIG), np.float32)
    d0[C:] = DEAD
    d0[np.arange(C), cy, cx] = 0.0
    idx = np.arange(P)
    Cp = np.zeros((P, P), np.float32); Cp[(idx + 1) % P, idx] = 1.0
    Cm = np.zeros((P, P), np.float32); Cm[(idx - 1) % P, idx] = 1.0
    wmats = np.concatenate([Cp, Cm], axis=1)           # (P, 224)
    dinits = []
    for k in range(NCORE):
        dl = _to_layout(d0[k * CPC:(k + 1) * CPC])     # (CPC,P,2,WS)
        dinits.append(np.ascontiguousarray(
            dl.transpose(1, 0, 2, 3).reshape(P, FREE_D)))
    return costs_in, wmats, dinits


def _legalize_waits(nc):
    """TRN2 TPB instructions carry a single sync-wait slot in the walrus
    codegen; hoist extra waits into standalone EventSemaphore instructions."""
    n_split = 0
    for blk in nc.main_func.blocks:
        new = []
        for inst in blk.instructions:
            si = getattr(inst, 'sync_info', None)
            ow = list(si.on_wait) if si is not None and si.on_wait else []
            if len(ow) > 1:
                for j, wsync in enumerate(ow[:-1]):
                    w = mybir.InstEventSemaphore(
                        name=f"{inst.name}_hw{j}",
                        engine=inst.engine, ins=[], outs=[],
                        sync_info=mybir.SyncInfo(on_wait=[wsync],
                                                 on_update=[]))
                    new.append(w)
                    n_split += 1
                si.on_wait = [ow[-1]]
            new.append(inst)
        blk.instructions[:] = new
    return n_split


# ------------------------------------------------------------ program build
def _build_program(n_iters=NUM_ITERS, legalize=True):
    nc = bass.Bass(use_seq_codegen=True)
    blob_cols = FREE_D + 2 * 4 * 2 * WS + 2 * P
    bin_ = nc.declare_dram_parameter("blob", [P, blob_cols], F32,
                                     isOutput=False)
    pout = nc.declare_dram_parameter("probs", [P, FREE_O], F32, isOutput=True)

    chunks = [(c0, min(c0 + CCH, CPC)) for c0 in range(0, CPC, CCH)]

    with tile.TileContext(nc) as tc:
        with (
            tc.tile_pool(name="persist", bufs=1) as pp,
            tc.tile_pool(name="psum", bufs=2, space="PSUM") as psp,
            tc.tile_pool(name="dram", bufs=1, space="DRAM") as dp,
        ):
            blob = pp.tile([P, FREE_D + 2 * 4 * 2 * WS + 2 * P], F32,
                           tag="blob")
            nc.sync.dma_start(blob[:], bin_[:])
            D = blob[:, 0:FREE_D]
            costs_sb = blob[:, FREE_D:FREE_D + 2 * 4 * 2 * WS]
            w_sb = blob[:, FREE_D + 2 * 4 * 2 * WS:]

            CpW = w_sb[:, 0:P]
            CmW = w_sb[:, P:2 * P]
            Dv = D.rearrange("p (c b w) -> p c b w", c=CPC, b=2)
            # cost view: (cr=2, d=4, b=2, ws)
            cv = costs_sb.rearrange("p (cr d b w) -> p cr d b w",
                                    cr=2, d=4, b=2)

            def cchunk(d, b_, n, w0, w1):
                # cost AP for n-cluster chunk, dir d: dims (n, [b], w)
                if b_ is None:
                    return cv[:, 0:n, d, :, w0:w1]
                return cv[:, 0:n, d, b_, w0:w1]

            with tc.tile_pool(name="chunk", bufs=2) as cp:
                for it in range(n_iters):
                    # wrap cols for all clusters: [0]<-[224], [225]<-[1]
                    nc.vector.tensor_copy(Dv[:, :, :, 0:1], Dv[:, :, :, W:W + 1])
                    nc.vector.tensor_copy(Dv[:, :, :, W + 1:W + 2],
                                          Dv[:, :, :, 1:2])
                    for (c0, c1) in chunks:
                        ncl = c1 - c0
                        nf = ncl * 2 * W          # interior cols this chunk
                        half = ncl * W
                        pt = psp.tile([P, 1024], F32, name=f"pt{it}_{c0}",
                                      tag="pt")
                        # circular shifts: slot0 = Cp @ D[b0], slot1 = Cm @ D[b1]
                        nc.tensor.matmul(pt[:, 0:half], CpW,
                                         Dv[:, c0:c1, 0, 1:W + 1],
                                         start=True, stop=True)
                        nc.tensor.matmul(pt[:, 512:512 + half], CmW,
                                         Dv[:, c0:c1, 1, 1:W + 1],
                                         start=True, stop=True)

                        a = [cp.tile([P, CCH * 2 * W], F32, name=f"a{i}_{it}_{c0}",
                                     tag=f"a{i}") for i in range(4)]
                        av = [t[:CCH and P, :nf].rearrange(
                            "p (c b w) -> p c b w", c=ncl, b=2) for t in a]
                        # dir0 (dy=-1): b0 <- (p,b1); b1 <- Cp-shifted b0
                        nc.vector.tensor_tensor(
                            av[0][:, :, 0, :], Dv[:, c0:c1, 1, 1:W + 1],
                            cchunk(0, 0, ncl, 1, W + 1), ALU.add)
                        nc.vector.tensor_tensor(
                            av[0][:, :, 1, :],
                            pt[:, 0:half].rearrange("p (c w) -> p c w", c=ncl),
                            cchunk(0, 1, ncl, 1, W + 1), ALU.add)
                        # dir1 (dy=+1): b1 <- (p,b0); b0 <- Cm-shifted b1
                        nc.vector.tensor_tensor(
                            av[1][:, :, 1, :], Dv[:, c0:c1, 0, 1:W + 1],
                            cchunk(1, 1, ncl, 1, W + 1), ALU.add)
                        nc.vector.tensor_tensor(
                            av[1][:, :, 0, :],
                            pt[:, 512:512 + half].rearrange(
                                "p (c w) -> p c w", c=ncl),
                            cchunk(1, 0, ncl, 1, W + 1), ALU.add)
                        # dir2 (dx=-1): w+1 ; dir3 (dx=+1): w-1
                        nc.vector.tensor_tensor(
                            av[2], Dv[:, c0:c1, :, 2:W + 2],
                            cchunk(2, None, ncl, 1, W + 1), ALU.add)
                        nc.gpsimd.tensor_tensor(
                            av[3], Dv[:, c0:c1, :, 0:W],
                            cchunk(3, None, ncl, 1, W + 1), ALU.add)

                        dmid = Dv[:, c0:c1, :, 1:W + 1]
                        m = cp.tile([P, CCH * 2 * W], F32, name=f"m{it}_{c0}",
                                    tag="m")
                        mf = m[:, :nf]
                        mv = mf.rearrange("p (c b w) -> p c b w", c=ncl, b=2)
                        nc.vector.tensor_tensor(mf, a[0][:, :nf], a[1][:, :nf],
                                                ALU.min)
                        nc.vector.tensor_tensor(mf, a[2][:, :nf], mf, ALU.min)
                        nc.vector.tensor_tensor(mf, a[3][:, :nf], mf, ALU.min)
                        nc.vector.tensor_tensor(mv, dmid, mv, ALU.min)

                        tcen = cp.tile([P, CCH * 2 * W], F32,
                                       name=f"tc{it}_{c0}", tag="tc")
                        nc.gpsimd.tensor_tensor(
                            tcen[:, :nf].rearrange("p (c b w) -> p c b w",
                                                   c=ncl, b=2),
                            dmid, mv, ALU.subtract)
                        for i in range(2):
                            nc.vector.tensor_tensor(a[i][:, :nf], a[i][:, :nf],
                                                    mf, ALU.subtract)
                        for i in range(2, 4):
                            nc.gpsimd.tensor_tensor(a[i][:, :nf], a[i][:, :nf],
                                                    mf, ALU.subtract)

                        e = [cp.tile([P, CCH * 2 * W], F16,
                                     name=f"e{i}_{it}_{c0}", tag=f"e{i}")
                             for i in range(5)]
                        for i in range(4):
                            nc.scalar.activation(e[i][:, :nf], a[i][:, :nf],
                                                 AF.Exp, scale=-1.0)
                        nc.scalar.activation(e[4][:, :nf], tcen[:, :nf],
                                             AF.Exp, scale=-1.0)

                        nc.vector.tensor_tensor(e[0][:, :nf], e[0][:, :nf],
                                                e[1][:, :nf], ALU.add)
                        nc.vector.tensor_tensor(e[2][:, :nf], e[2][:, :nf],
                                                e[3][:, :nf], ALU.add)
                        nc.vector.tensor_tensor(e[0][:, :nf], e[0][:, :nf],
                                                e[2][:, :nf], ALU.add)
                        nc.vector.tensor_tensor(e[0][:, :nf], e[0][:, :nf],
                                                e[4][:, :nf], ALU.add)

                        L = cp.tile([P, CCH * 2 * W], F32, name=f"L{it}_{c0}",
                                    tag="L")
                        nc.scalar.activation(L[:, :nf], e[0][:, :nf], AF.Ln)

                        nc.vector.scalar_tensor_tensor(
                            dmid,
                            L[:, :nf].rearrange("p (c b w) -> p c b w",
                                                c=ncl, b=2),
                            -1.0, mv, ALU.mult, ALU.add)

            # ---------------- softmax over all CPAD clusters (2 AllReduces)
            with tc.tile_pool(name="smax", bufs=1) as sp:
                u_sb = sp.tile([P, FREE_O], F32, tag="u")
                uv = u_sb[:].rearrange("p (c b w) -> p c b w", c=CPC, b=2)
                gmin = sp.tile([P, 448], F32, tag="gmin")
                g2 = gmin[:].rearrange("p (b w) -> p b w", b=2)
                nc.vector.tensor_tensor(g2, Dv[:, 0, :, 1:W + 1],
                                        Dv[:, 1, :, 1:W + 1], ALU.min)
                for c in range(2, CPC):
                    nc.vector.tensor_tensor(g2, Dv[:, c, :, 1:W + 1], g2,
                                            ALU.min)

                cc_in = dp.tile([P, 448], F32, tag="ccin")
                cc_out = dp.tile([P, 448], F32, tag="ccout",
                                 addr_space="Shared")
                nc.sync.dma_start(cc_in[:], gmin[:])
                nc.gpsimd.collective_compute(
                    "AllReduce", ALU.min,
                    replica_groups=[list(range(NCORE))],
                    ins=[cc_in[:].opt()], outs=[cc_out[:].opt()])
                nc.sync.dma_start(gmin[:], cc_out[:])

                for c in range(CPC):
                    nc.vector.tensor_tensor(uv[:, c], Dv[:, c, :, 1:W + 1],
                                            g2, ALU.subtract)
                nc.scalar.activation(u_sb[:], u_sb[:], AF.Exp,
                                     scale=-1.0 / ALPHA)

                ssum = sp.tile([P, 448], F32, tag="ssum")
                nc.vector.tensor_reduce(
                    ssum[:], u_sb[:].rearrange("p (c x) -> p x c", c=CPC),
                    mybir.AxisListType.X, ALU.add)

                cc_in2 = dp.tile([P, 448], F32, tag="ccin2")
                cc_out2 = dp.tile([P, 448], F32, tag="ccout2",
                                  addr_space="Shared")
                nc.sync.dma_start(cc_in2[:], ssum[:])
                nc.gpsimd.collective_compute(
                    "AllReduce", ALU.add,
                    replica_groups=[list(range(NCORE))],
                    ins=[cc_in2[:].opt()], outs=[cc_out2[:].opt()])
                nc.sync.dma_start(ssum[:], cc_out2[:])

                rec = sp.tile([P, 448], F32, tag="rec")
                nc.vector.reciprocal(rec[:], ssum[:])
                for c in range(CPC):
                    nc.vector.tensor_tensor(
                        u_sb[:, c * 448:(c + 1) * 448],
                        u_sb[:, c * 448:(c + 1) * 448], rec[:], ALU.mult)

                nc.sync.dma_start(pout[:], u_sb[:])
    if legalize:
        _legalize_waits(nc)
    return nc


def _get_program(n_iters=NUM_ITERS):
    global _PROG
    if _PROG is None or _PROG[0] != n_iters:
        _PROG = (n_iters, _build_program(n_iters))
    return _PROG[1]


# ------------------------------------------------------------------ entry
def kernel(x, edge_weight, int_cy, int_cx, _trace=False, _n_iters=NUM_ITERS):
    x = np.asarray(x); edge_weight = np.asarray(edge_weight)
    int_cy = np.asarray(int_cy); int_cx = np.asarray(int_cx)
    assert x.shape == (B, 3, H, W) and int_cy.shape == (C,)

    costs_in, wmats, dinits = _host_inputs(x, edge_weight, int_cy, int_cx)
    nc = _get_program(_n_iters)
    in_maps = [{"blob": np.ascontiguousarray(
        np.concatenate([dinits[k], costs_in, wmats], axis=1))}
               for k in range(NCORE)]
    res = run_bass_kernel_spmd(nc, in_maps, list(range(NCORE)), trace=_trace)

    out = np.zeros((CPAD, H, W), np.float32)
    for k in range(NCORE):
        pr = res.results[k]["probs"].reshape(P, CPC, 2, W)
        out[k * CPC:(k + 1) * CPC] = (
            pr.transpose(1, 0, 2, 3).reshape(CPC, H, W))
    full = out[None, :C].astype(np.float32)
    if _trace:
        return full, res
    return full


# revision 13
# speedup vs baseline: 1.0089x; 1.0089x over previous
"""Differentiable Voronoi propagation on 8 Trainium2 NeuronCores.

Strategy (C-sharded): 196 clusters padded to 200, 25 per core. Each core
propagates its distance maps D = ALPHA*dist for 20 iterations fully
SBUF-resident, then the softmax over all 200 clusters is computed with two
small AllReduces (per-pixel min, per-pixel sum).

Per-core layout: image row r = 2p + b  (p in [0,112) partitions, b in {0,1});
SBUF free dim = (cluster c, parity b, stored col ws) with ws = [wrapL, w0..w223,
wrapR] so E/W rolls are free-dim offsets. N/S rolls decompose into a free-dim
parity swap plus a circular partition shift done as an exact fp32 permutation
matmul on the TensorEngine (PSUM), everything else on VectorE/GpSimd/ScalarE:
cand_i = roll_i(D)+10*cost_i, m = chained min, e_i = Exp(-(cand_i-m)) in fp16,
D = m - Ln(sum e_i).
"""
import numpy as np

import concourse.bass as bass
import concourse.mybir as mybir
import concourse.tile as tile
from concourse.bass_utils import run_bass_kernel_spmd

B, C, H, W = 1, 196, 224, 224
NUM_ITERS = 20
ALPHA = 10.0
BIG = 1.0e4
DEAD = 3.0e5
CPAD, NCORE = 200, 8
CPC = CPAD // NCORE      # 25 clusters per core
P = 112                  # partitions (= H/2)
WS = W + 2               # stored row width incl wrap cols
FREE_D = CPC * 2 * WS    # 11300
FREE_O = CPC * 2 * W     # 11200
CCH = 2                  # clusters per chunk
DIRS = ((-1, 0), (1, 0), (0, -1), (0, 1))

AF = mybir.ActivationFunctionType
ALU = mybir.AluOpType
F32 = mybir.dt.float32
F16 = mybir.dt.float16

_PROG = None


# ----------------------------------------------------------------- host prep
def _host_costs(x, edge_weight):
    x = np.asarray(x, np.float32)
    ew = np.asarray(edge_weight, np.float32)
    gray = (0.2989 * x[:, 0] + 0.587 * x[:, 1] + 0.114 * x[:, 2])[0]
    gp = np.pad(gray, 1)
    edges = np.zeros((2, H, W), np.float32)
    for o in range(2):
        k = ew[o, 0]
        acc = np.zeros((H, W), np.float32)
        for i in range(3):
            for j in range(3):
                acc += k[i, j] * gp[i:i + H, j:j + W]
        edges[o] = acc
    grad = np.sqrt(edges[0] ** 2 + edges[1] ** 2)
    wgrad = grad ** 4 * np.float32(10.0)
    costs = np.zeros((4, H, W), np.float32)
    for i, (dy, dx) in enumerate(DIRS):
        rolled = np.roll(x[0], (dy, dx), axis=(1, 2))
        costs[i] = wgrad + np.float32(10.0) * np.abs(x[0] - rolled).sum(0)
    return (np.float32(ALPHA) * costs).astype(np.float32)


def _to_layout(plane):
    """(..., H, W) -> (..., P, 2, WS) with wrap cols; row r = 2p + b."""
    sh = plane.shape[:-2]
    out = np.zeros(sh + (P, 2, WS), plane.dtype)
    r = plane.reshape(sh + (P, 2, W))
    out[..., 1:W + 1] = r
    out[..., 0] = r[..., W - 1]
    out[..., W + 1] = r[..., 0]
    return out


def _host_inputs(x, edge_weight, int_cy, int_cx):
    costs10 = _host_costs(x, edge_weight)
    costs_l = _to_layout(costs10)                      # (4,P,2,WS)
    # replicate over a 2-cluster axis: layout [P, (cr=2, d=4, b=2, ws)]
    rep = np.broadcast_to(costs_l.transpose(1, 0, 2, 3)[:, None],
                          (P, 2, 4, 2, WS))
    costs_in = np.ascontiguousarray(rep.reshape(P, 2 * 4 * 2 * WS))
    cy = np.asarray(int_cy); cx = np.asarray(int_cx)
    d0 = np.full((CPAD, H, W), np.float32(ALPHA * BIG), np.float32)
    d0[C:] = DEAD
    d0[np.arange(C), cy, cx] = 0.0
    idx = np.arange(P)
    Cp = np.zeros((P, P), np.float32); Cp[(idx + 1) % P, idx] = 1.0
    Cm = np.zeros((P, P), np.float32); Cm[(idx - 1) % P, idx] = 1.0
    wmats = np.concatenate([Cp, Cm], axis=1)           # (P, 224)
    dinits = []
    for k in range(NCORE):
        dl = _to_layout(d0[k * CPC:(k + 1) * CPC])     # (CPC,P,2,WS)
        dinits.append(np.ascontiguousarray(
            dl.transpose(1, 0, 2, 3).reshape(P, FREE_D)))
    return costs_in, wmats, dinits


def _legalize_waits(nc):
    """TRN2 TPB instructions carry a single sync-wait slot in the walrus
    codegen; hoist extra waits into standalone EventSemaphore instructions."""
    n_split = 0
    for blk in nc.main_func.blocks:
        new = []
        for inst in blk.instructions:
            si = getattr(inst, 'sync_info', None)
            ow = list(si.on_wait) if si is not None and si.on_wait else []
            if len(ow) > 1:
                for j, wsync in enumerate(ow[:-1]):
                    w = mybir.InstEventSemaphore(
                        name=f"{inst.name}_hw{j}",
                        engine=inst.engine, ins=[], outs=[],
                        sync_info=mybir.SyncInfo(on_wait=[wsync],
                                                 on_update=[]))
                    new.append(w)
                    n_split += 1
                si.on_wait = [ow[-1]]
            new.append(inst)
        blk.instructions[:] = new
    return n_split


# ------------------------------------------------------------ program build
def _build_program(n_iters=NUM_ITERS, legalize=True):
    nc = bass.Bass(use_seq_codegen=True)
    blob_cols = FREE_D + 2 * 4 * 2 * WS + 2 * P
    bin_ = nc.declare_dram_parameter("blob", [P, blob_cols], F32,
                                     isOutput=False)
    pout = nc.declare_dram_parameter("probs", [P, FREE_O], F32, isOutput=True)

    chunks = [(c0, min(c0 + CCH, CPC)) for c0 in range(0, CPC, CCH)]

    with tile.TileContext(nc) as tc:
        with (
            tc.tile_pool(name="persist", bufs=1) as pp,
            tc.tile_pool(name="psum", bufs=4, space="PSUM") as psp,
            tc.tile_pool(name="dram", bufs=1, space="DRAM") as dp,
        ):
            blob = pp.tile([P, FREE_D + 2 * 4 * 2 * WS + 2 * P], F32,
                           tag="blob")
            nc.sync.dma_start(blob[:], bin_[:])
            D = blob[:, 0:FREE_D]
            costs_sb = blob[:, FREE_D:FREE_D + 2 * 4 * 2 * WS]
            w_sb = blob[:, FREE_D + 2 * 4 * 2 * WS:]

            CpW = w_sb[:, 0:P]
            CmW = w_sb[:, P:2 * P]
            Dv = D.rearrange("p (c b w) -> p c b w", c=CPC, b=2)
            # cost view: (cr=2, d=4, b=2, ws)
            cv = costs_sb.rearrange("p (cr d b w) -> p cr d b w",
                                    cr=2, d=4, b=2)

            def cchunk(d, b_, n, w0, w1):
                # cost AP for n-cluster chunk, dir d: dims (n, [b], w)
                if b_ is None:
                    return cv[:, 0:n, d, :, w0:w1]
                return cv[:, 0:n, d, b_, w0:w1]

            with tc.tile_pool(name="chunk", bufs=2) as cp:
                for it in range(n_iters):
                    for (c0, c1) in chunks:
                        # wrap cols for this chunk: [0]<-[224], [225]<-[1]
                        nc.vector.tensor_copy(Dv[:, c0:c1, :, 0:1],
                                              Dv[:, c0:c1, :, W:W + 1])
                        nc.vector.tensor_copy(Dv[:, c0:c1, :, W + 1:W + 2],
                                              Dv[:, c0:c1, :, 1:2])
                        ncl = c1 - c0
                        nf = ncl * 2 * W          # interior cols this chunk
                        half = ncl * W
                        pt = psp.tile([P, 1024], F32, name=f"pt{it}_{c0}",
                                      tag="pt")
                        # circular shifts: slot0 = Cp @ D[b0], slot1 = Cm @ D[b1]
                        nc.tensor.matmul(pt[:, 0:half], CpW,
                                         Dv[:, c0:c1, 0, 1:W + 1],
                                         start=True, stop=True)
                        nc.tensor.matmul(pt[:, 512:512 + half], CmW,
                                         Dv[:, c0:c1, 1, 1:W + 1],
                                         start=True, stop=True)

                        a = [cp.tile([P, CCH * 2 * W], F32, name=f"a{i}_{it}_{c0}",
                                     tag=f"a{i}") for i in range(4)]
                        av = [t[:CCH and P, :nf].rearrange(
                            "p (c b w) -> p c b w", c=ncl, b=2) for t in a]
                        # dir0 (dy=-1): b0 <- (p,b1); b1 <- Cp-shifted b0
                        nc.vector.tensor_tensor(
                            av[0][:, :, 0, :], Dv[:, c0:c1, 1, 1:W + 1],
                            cchunk(0, 0, ncl, 1, W + 1), ALU.add)
                        nc.vector.tensor_tensor(
                            av[0][:, :, 1, :],
                            pt[:, 0:half].rearrange("p (c w) -> p c w", c=ncl),
                            cchunk(0, 1, ncl, 1, W + 1), ALU.add)
                        # dir1 (dy=+1): b1 <- (p,b0); b0 <- Cm-shifted b1
                        nc.vector.tensor_tensor(
                            av[1][:, :, 1, :], Dv[:, c0:c1, 0, 1:W + 1],
                            cchunk(1, 1, ncl, 1, W + 1), ALU.add)
                        nc.vector.tensor_tensor(
                            av[1][:, :, 0, :],
                            pt[:, 512:512 + half].rearrange(
                                "p (c w) -> p c w", c=ncl),
                            cchunk(1, 0, ncl, 1, W + 1), ALU.add)
                        # dir2 (dx=-1): w+1 ; dir3 (dx=+1): w-1
                        nc.vector.tensor_tensor(
                            av[2], Dv[:, c0:c1, :, 2:W + 2],
                            cchunk(2, None, ncl, 1, W + 1), ALU.add)
                        nc.gpsimd.tensor_tensor(
                            av[3], Dv[:, c0:c1, :, 0:W],
                            cchunk(3, None, ncl, 1, W + 1), ALU.add)

                        dmid = Dv[:, c0:c1, :, 1:W + 1]
                        m = cp.tile([P, CCH * 2 * W], F32, name=f"m{it}_{c0}",
                                    tag="m")
                        mf = m[:, :nf]
                        mv = mf.rearrange("p (c b w) -> p c b w", c=ncl, b=2)
                        nc.vector.tensor_tensor(mf, a[0][:, :nf], a[1][:, :nf],
                                                ALU.min)
                        nc.vector.tensor_tensor(mf, a[2][:, :nf], mf, ALU.min)
                        nc.vector.tensor_tensor(mf, a[3][:, :nf], mf, ALU.min)
                        nc.vector.tensor_tensor(mv, dmid, mv, ALU.min)

                        tcen = cp.tile([P, CCH * 2 * W], F32,
                                       name=f"tc{it}_{c0}", tag="tc")
                        nc.gpsimd.tensor_tensor(
                            tcen[:, :nf].rearrange("p (c b w) -> p c b w",
                                                   c=ncl, b=2),
                            dmid, mv, ALU.subtract)
                        for i in range(2):
                            nc.vector.tensor_tensor(a[i][:, :nf], a[i][:, :nf],
                                                    mf, ALU.subtract)
                        for i in range(2, 4):
                            nc.gpsimd.tensor_tensor(a[i][:, :nf], a[i][:, :nf],
                                                    mf, ALU.subtract)

                        e = [cp.tile([P, CCH * 2 * W], F16,
                                     name=f"e{i}_{it}_{c0}", tag=f"e{i}")
                             for i in range(5)]
                        for i in range(4):
                            nc.scalar.activation(e[i][:, :nf], a[i][:, :nf],
                                                 AF.Exp, scale=-1.0)
                        nc.scalar.activation(e[4][:, :nf], tcen[:, :nf],
                                             AF.Exp, scale=-1.0)

                        nc.vector.tensor_tensor(e[0][:, :nf], e[0][:, :nf],
                                                e[1][:, :nf], ALU.add)
                        nc.vector.tensor_tensor(e[2][:, :nf], e[2][:, :nf],
                                                e[3][:, :nf], ALU.add)
                        nc.vector.tensor_tensor(e[0][:, :nf], e[0][:, :nf],
                                                e[2][:, :nf], ALU.add)
                        nc.vector.tensor_tensor(e[0][:, :nf], e[0][:, :nf],
                                                e[4][:, :nf], ALU.add)

                        L = cp.tile([P, CCH * 2 * W], F32, name=f"L{it}_{c0}",
                                    tag="L")
                        nc.scalar.activation(L[:, :nf], e[0][:, :nf], AF.Ln)

                        nc.vector.scalar_tensor_tensor(
                            dmid,
                            L[:, :nf].rearrange("p (c b w) -> p c b w",
                                                c=ncl, b=2),
                            -1.0, mv, ALU.mult, ALU.add)

            # ---------------- softmax over all CPAD clusters (2 AllReduces)
            with tc.tile_pool(name="smax", bufs=1) as sp:
                u_sb = sp.tile([P, FREE_O], F32, tag="u")
                uv = u_sb[:].rearrange("p (c b w) -> p c b w", c=CPC, b=2)
                gmin = sp.tile([P, 448], F32, tag="gmin")
                g2 = gmin[:].rearrange("p (b w) -> p b w", b=2)
                nc.vector.tensor_tensor(g2, Dv[:, 0, :, 1:W + 1],
                                        Dv[:, 1, :, 1:W + 1], ALU.min)
                for c in range(2, CPC):
                    nc.vector.tensor_tensor(g2, Dv[:, c, :, 1:W + 1], g2,
                                            ALU.min)

                cc_in = dp.tile([P, 448], F32, tag="ccin")
                cc_out = dp.tile([P, 448], F32, tag="ccout",
                                 addr_space="Shared")
                nc.sync.dma_start(cc_in[:], gmin[:])
                nc.gpsimd.collective_compute(
                    "AllReduce", ALU.min,
                    replica_groups=[list(range(NCORE))],
                    ins=[cc_in[:].opt()], outs=[cc_out[:].opt()])
                nc.sync.dma_start(gmin[:], cc_out[:])

                for c in range(CPC):
                    nc.vector.tensor_tensor(uv[:, c], Dv[:, c, :, 1:W + 1],
                                            g2, ALU.subtract)
                nc.scalar.activation(u_sb[:], u_sb[:], AF.Exp,
                                     scale=-1.0 / ALPHA)

                ssum = sp.tile([P, 448], F32, tag="ssum")
                nc.vector.tensor_reduce(
                    ssum[:], u_sb[:].rearrange("p (c x) -> p x c", c=CPC),
                    mybir.AxisListType.X, ALU.add)

                cc_in2 = dp.tile([P, 448], F32, tag="ccin2")
                cc_out2 = dp.tile([P, 448], F32, tag="ccout2",
                                  addr_space="Shared")
                nc.sync.dma_start(cc_in2[:], ssum[:])
                nc.gpsimd.collective_compute(
                    "AllReduce", ALU.add,
                    replica_groups=[list(range(NCORE))],
                    ins=[cc_in2[:].opt()], outs=[cc_out2[:].opt()])
                nc.sync.dma_start(ssum[:], cc_out2[:])

                rec = sp.tile([P, 448], F32, tag="rec")
                nc.vector.reciprocal(rec[:], ssum[:])
                for c in range(CPC):
                    nc.vector.tensor_tensor(
                        u_sb[:, c * 448:(c + 1) * 448],
                        u_sb[:, c * 448:(c + 1) * 448], rec[:], ALU.mult)

                nc.sync.dma_start(pout[:], u_sb[:])
    if legalize:
        _legalize_waits(nc)
    return nc


def _get_program(n_iters=NUM_ITERS):
    global _PROG
    if _PROG is None or _PROG[0] != n_iters:
        _PROG = (n_iters, _build_program(n_iters))
    return _PROG[1]


# ------------------------------------------------------------------ entry
def kernel(x, edge_weight, int_cy, int_cx, _trace=False, _n_iters=NUM_ITERS):
    x = np.asarray(x); edge_weight = np.asarray(edge_weight)
    int_cy = np.asarray(int_cy); int_cx = np.asarray(int_cx)
    assert x.shape == (B, 3, H, W) and int_cy.shape == (C,)

    costs_in, wmats, dinits = _host_inputs(x, edge_weight, int_cy, int_cx)
    nc = _get_program(_n_iters)
    in_maps = [{"blob": np.ascontiguousarray(
        np.concatenate([dinits[k], costs_in, wmats], axis=1))}
               for k in range(NCORE)]
    res = run_bass_kernel_spmd(nc, in_maps, list(range(NCORE)), trace=_trace)

    out = np.zeros((CPAD, H, W), np.float32)
    for k in range(NCORE):
        pr = res.results[k]["probs"].reshape(P, CPC, 2, W)
        out[k * CPC:(k + 1) * CPC] = (
            pr.transpose(1, 0, 2, 3).reshape(CPC, H, W))
    full = out[None, :C].astype(np.float32)
    if _trace:
        return full, res
    return full
